# revision 9
# baseline (speedup 1.0000x reference)
"""DechirpSTFT Trainium2 kernel.

Math: the reference pipeline (hann window -> per-chirp lerp resample * jac
-> rfft(1024)) is linear in the windowed signal, so it folds into one
per-chirp matrix G_d[k, f2] (k = sample within window, f2 = interleaved
re/im of the 513 rfft bins).  The device kernel is then a dense matmul

    out[row, f2] = sum_k x[b, 512*w + k] * G_d[k, f2]

with rows = (b, w) on PSUM partitions, k contracted on the PE partitions
(8 chunks of 128), f2 streamed (3 chunks of 342 <= 512 per PSUM bank).
G is computed host-side from dlnf (float64 accumulation of twiddles).

Sharding: D=16 chirp rates, 2 per core across 8 cores (x replicated,
G sliced per core).  No cross-core communication.

Layouts shipped from host:
  xt : (128, B, 2048)  f32   xt[p, b, m] = x[b, 128*m + p]
       so lhsT tile [k_p, w] for (b, kc) is xt[:, b, 4*w + kc] - a step-4
       free-axis slice (window element k = 128*kc + k_p, sample index
       512*w + k = 128*(4*w + kc) + k_p).
  g  : (128, 2, 8, 1026) f32  g[p, d, kc, f2] = G_d[128*kc + p, f2]
  out: (B, 511, 2, 1026) f32 per core -> complex64 (B, 511, 2, 513)
"""

import os
import sys

sys.path.insert(0, "/opt/trn_rl_repo")

import numpy as np

# ---- problem constants (hardcoded; kernel.py must be self-contained) ----
B = 4
N = 262144
K = 1024
HOP = 512
NW = (N - K) // HOP + 1          # 511
KTAU = 1024
NF = 513                         # rfft bins
FW = 2 * NF                      # 1026 interleaved re/im
D = 16
NCORES = 8
D_PER = D // NCORES              # 2
KC = 8                           # contraction chunks of 128
MQ = N // 512                    # 512  (m = 4*w + kc, w < 512)
FCH = 342                        # f2 chunk (3 * 342 = 1026)
NFCH = 3
RC_SIZES = (128, 128, 128, 127)  # 511 rows per batch
EPS = 1e-8

_MM_DTYPE = os.environ.get("KERNEL_MM_DTYPE", "float32r")

_cache = {}


# --------------------------------------------------------------------------
# host-side G construction
# --------------------------------------------------------------------------
def _build_tables_np(dlnf):
    """Reference's per-chirp tables in numpy float32 (fallback path)."""
    dlnf = dlnf.astype(np.float32)
    beta = (2.0 * dlnf).astype(np.float32)
    small = np.abs(beta) < EPS
    beta_safe = np.where(small, np.float32(EPS), beta).astype(np.float32)
    e2b = np.exp(2.0 * beta_safe).astype(np.float32)

    tau = (2.0 * np.arange(KTAU, dtype=np.float32) / KTAU - 1.0).astype(np.float32)
    t_source = np.log(
        1.0 + (tau[None, :] + 1.0) / 2.0 * (e2b[:, None] - 1.0)
    ).astype(np.float32)
    t_source = (t_source / beta_safe[:, None] - 1.0).astype(np.float32)
    t_source = np.where(small[:, None], tau[None, :], t_source)

    tau_mid = np.float32(2.0 * (KTAU // 2) / KTAU - 1.0)
    t_mid = (
        np.log(1.0 + (tau_mid + 1.0) / 2.0 * (e2b - 1.0)) / beta_safe - 1.0
    ).astype(np.float32)
    t_mid = np.where(small, tau_mid, t_mid)

    jac = np.exp(-beta_safe[:, None] * (t_source - t_mid[:, None])).astype(np.float32)
    jac = np.where(small[:, None], np.float32(1.0), jac)

    idx = (np.float32(K / 2.0) * (t_source + 1.0)).astype(np.float32)
    idx_lo = np.clip(idx.astype(np.int32), 0, K - 2)
    frac = (idx - idx_lo.astype(np.float32)).astype(np.float32)
    return idx_lo, frac, jac


def _build_tables(dlnf):
    """Per-chirp tables, computed with jax on the CPU backend so the f32
    transcendentals (log/exp) match the reference bit-for-bit."""
    try:
        import jax
        import jax.numpy as jnp

        cpu = jax.devices("cpu")[0]
    except Exception:
        return _build_tables_np(dlnf)

    with jax.default_device(cpu):
        beta = 2.0 * jnp.asarray(dlnf, dtype=jnp.float32)
        small = jnp.abs(beta) < EPS
        beta_safe = jnp.where(small, EPS, beta)
        e2b = jnp.exp(2.0 * beta_safe)

        tau = 2.0 * jnp.arange(KTAU, dtype=jnp.float32) / KTAU - 1.0
        t_source = (
            jnp.log(1.0 + (tau[None, :] + 1.0) / 2.0 * (e2b[:, None] - 1.0))
            / beta_safe[:, None]
            - 1.0
        )
        t_source = jnp.where(small[:, None], tau[None, :], t_source)

        tau_mid = 2.0 * (KTAU // 2) / KTAU - 1.0
        t_mid = (
            jnp.log(1.0 + (tau_mid + 1.0) / 2.0 * (e2b - 1.0)) / beta_safe - 1.0
        )
        t_mid = jnp.where(small, tau_mid, t_mid)

        jac = jnp.exp(-beta_safe[:, None] * (t_source - t_mid[:, None]))
        jac = jnp.where(small[:, None], 1.0, jac)

        idx = (K / 2.0) * (t_source + 1.0)
        idx_lo = jnp.clip(idx.astype(jnp.int32), 0, K - 2)
        frac = idx - idx_lo.astype(jnp.float32)
    return np.asarray(idx_lo), np.asarray(frac), np.asarray(jac)


def _build_G(dlnf):
    """G[d, k, f2] f32: fused hann * lerp-resample * jac * rfft operator."""
    nd = dlnf.shape[0]
    idx_lo, frac, jac = _build_tables(dlnf)
    t = np.arange(KTAU, dtype=np.float64)
    f = np.arange(NF, dtype=np.float64)
    ang = 2.0 * np.pi * np.outer(t, f) / KTAU
    Wre = np.cos(ang)
    Wim = -np.sin(ang)
    n = np.arange(K, dtype=np.float32)
    hann = (0.5 * (1.0 - np.cos(2.0 * np.pi * n / K))).astype(np.float32)

    G = np.zeros((nd, K, FW), dtype=np.float64)
    for d in range(nd):
        c_lo = (jac[d] * (1.0 - frac[d])).astype(np.float64)
        c_hi = (jac[d] * frac[d]).astype(np.float64)
        Gre = np.zeros((K, NF))
        Gim = np.zeros((K, NF))
        np.add.at(Gre, idx_lo[d], c_lo[:, None] * Wre)
        np.add.at(Gim, idx_lo[d], c_lo[:, None] * Wim)
        np.add.at(Gre, idx_lo[d] + 1, c_hi[:, None] * Wre)
        np.add.at(Gim, idx_lo[d] + 1, c_hi[:, None] * Wim)
        G[d, :, 0::2] = Gre
        G[d, :, 1::2] = Gim
    G *= hann[None, :, None].astype(np.float64)
    G[:, :, FW - 1] = 0.0  # rfft Nyquist imag is exactly 0
    return G.astype(np.float32)


# --------------------------------------------------------------------------
# device program
# --------------------------------------------------------------------------
def _build_nc(iters=1):
    import concourse.bass as bass
    import concourse.bacc as bacc
    import concourse.mybir as mybir
    from concourse import tile

    mm_dt = {
        "float32r": mybir.dt.float32r,
        "float32": mybir.dt.float32,
        "bfloat16": mybir.dt.bfloat16,
    }[_MM_DTYPE]
    f32 = mybir.dt.float32

    nc = bacc.Bacc("TRN2", target_bir_lowering=False, debug=False)

    xt_d = nc.dram_tensor("xt", [128, B, MQ, 4], mm_dt, kind="ExternalInput")
    g_d = nc.dram_tensor("g", [128, D_PER, KC, FW], mm_dt, kind="ExternalInput")
    out_d = nc.dram_tensor("out", [B, NW, D_PER, FW], f32, kind="ExternalOutput")

    def body(nc, tc, xpool, gpool, spool, ppool):
        x_sb = xpool.tile([128, B, MQ, 4], mm_dt, name="x_sb")
        g_sb = gpool.tile([128, D_PER, KC, FW], mm_dt, name="g_sb")

        # split loads so the first matmuls don't wait for everything
        for b in range(B):
            nc.sync.dma_start(x_sb[:, b], xt_d[:, b])
        for fc in range(NFCH):
            fs = slice(fc * FCH, (fc + 1) * FCH)
            nc.sync.dma_start(g_sb[:, :, :, fs], g_d[:, :, :, fs])

        for b in range(B):
            w0 = 0
            for rc, cnt in enumerate(RC_SIZES):
                st = spool.tile([128, D_PER, FW], f32, name="st")
                for fc in range(NFCH):
                    fs = slice(fc * FCH, (fc + 1) * FCH)
                    ps = [
                        ppool.tile([128, FCH], f32, name="ps", tag="ps")
                        for _ in range(D_PER)
                    ]
                    for kc in range(KC):
                        # window w, chunk kc -> m = 4*w + kc
                        q, r = divmod(kc, 4)
                        lhsT = x_sb[:, b, w0 + q : w0 + q + cnt, r]
                        for d in range(D_PER):
                            nc.tensor.matmul(
                                ps[d][:cnt],
                                lhsT,
                                g_sb[:, d, kc, fs],
                                start=(kc == 0),
                                stop=(kc == KC - 1),
                            )
                    nc.vector.tensor_copy(st[:cnt, 0, fs], ps[0][:cnt])
                    nc.scalar.copy(st[:cnt, 1, fs], ps[1][:cnt])
                nc.sync.dma_start(out_d[b, w0 : w0 + cnt], st[:cnt])
                w0 += cnt

    with tile.TileContext(nc) as tc:
        with (
            tc.tile_pool(name="xsb", bufs=1) as xpool,
            tc.tile_pool(name="gsb", bufs=1) as gpool,
            tc.tile_pool(name="stage", bufs=3) as spool,
            tc.tile_pool(name="psum", bufs=8, space="PSUM") as ppool,
        ):
            for _ in range(iters):
                body(nc, tc, xpool, gpool, spool, ppool)

    nc.compile()
    return nc


def _get_nc(iters=1):
    key = ("nc", iters)
    if key not in _cache:
        _cache[key] = _build_nc(iters)
    return _cache[key]


# --------------------------------------------------------------------------
# entry point
# --------------------------------------------------------------------------
def kernel(x, dlnf, n_hann_splits=1, **_unused):
    from concourse.bass_utils import run_bass_kernel_spmd

    x = np.asarray(x, dtype=np.float32)
    dlnf = np.asarray(dlnf, dtype=np.float32)

    G = _build_G(dlnf)                                     # (16, 1024, 1026)
    # g layout per core: [128, D_PER, KC, FW]
    g_all = np.ascontiguousarray(
        G.reshape(D, KC, 128, FW).transpose(2, 0, 1, 3)    # (128, 16, 8, FW)
    )
    # xt layout: [128, B, MQ, 4]; xt[p, b, mq, r] = x[b, 128*(4*mq+r) + p]
    xt = np.ascontiguousarray(
        x.reshape(B, MQ, 4, 128).transpose(3, 0, 1, 2)
    )

    iters = int(os.environ.get("KERNEL_ITERS", "1"))
    nc = _get_nc(iters)
    in_maps = [
        {"xt": xt, "g": np.ascontiguousarray(g_all[:, c * D_PER : (c + 1) * D_PER])}
        for c in range(NCORES)
    ]
    trace = bool(int(os.environ.get("KERNEL_TRACE", "0")))
    res = run_bass_kernel_spmd(
        nc, in_maps, core_ids=list(range(NCORES)), trace=trace
    )
    _cache["last_result"] = res

    # assemble: per-core (B, NW, D_PER, FW) f32 -> (B, NW, D, NF) c64
    per_core = [
        r["out"].view(np.complex64).reshape(B, NW, D_PER, NF) for r in res.results
    ]
    out = np.concatenate(per_core, axis=2)
    return out


if __name__ == "__main__":
    rng = np.random.default_rng(0)
    x = rng.standard_normal((B, N), dtype=np.float32)
    dlnf = rng.uniform(-0.5, 0.5, size=(D,)).astype(np.float32)
    out = kernel(x, dlnf, 1)
    print("out:", out.shape, out.dtype)


# revision 17
# speedup vs baseline: 71.7258x; 71.7258x over previous
"""DechirpSTFT Trainium2 kernel.

Math: the reference pipeline (hann window -> per-chirp lerp resample * jac
-> rfft(1024)) is linear in the windowed signal, so it folds into one
per-chirp matrix G_d[k, f2] (k = sample within window, f2 = interleaved
re/im of the 513 rfft bins).  The device kernel is then a dense matmul

    out[row, f2] = sum_k x[b, 512*w + k] * G_d[k, f2]

with rows = (b, w) on PSUM partitions, k contracted on the PE partitions
(8 chunks of 128), f2 streamed (3 chunks of 342 <= 512 per PSUM bank).
G is computed host-side from dlnf (float64 accumulation of twiddles).

Sharding: D=16 chirp rates, 2 per core across 8 cores (x replicated,
G sliced per core).  No cross-core communication.

Layouts shipped from host (matmul operand dtype = KERNEL_MM_DTYPE,
default float32r = fp32 bits, relaxed PE compute at bf16 column rate):
  xt : (128, B, 512, 4)      xt[p, b, mq, r] = x[b, 128*(4*mq+r) + p]
       lhsT tile [k_p, w] for (b, kc) is xt[:, b, w + kc//4, kc%4] over a
       step-4 free-axis slice (window element k = 128*kc + k_p, sample
       index 512*w + k = 128*(4*w + kc) + k_p).
  g  : (128, 2, 8, 1026)     g[p, d, kc, f2] = G_d[128*kc + p, f2]
  out: (B, 511, 2, 1026) f32 per core -> complex64 (B, 511, 2, 513)

Timing note: no NTFF profiling is reachable through this axon client, so
test.py measures HW time as (wall(For_i(T)) - wall(For_i(1))) / (T-1)
with the jitted executable and device-resident inputs held across calls.
"""

import os
import sys

sys.path.insert(0, "/opt/trn_rl_repo")

import numpy as np

# ---- problem constants (hardcoded; kernel.py must be self-contained) ----
B = 4
N = 262144
K = 1024
HOP = 512
NW = (N - K) // HOP + 1          # 511
KTAU = 1024
NF = 513                         # rfft bins
FW = 2 * NF                      # 1026 interleaved re/im
D = 16
NCORES = 8
D_PER = D // NCORES              # 2
KC = 8                           # contraction chunks of 128
MQ = N // 512                    # 512  (m = 4*w + kc, w < 512)
FCH = 342                        # f2 chunk (3 * 342 = 1026)
NFCH = 3
RC_SIZES = (128, 128, 128, 127)  # 511 rows per batch
EPS = 1e-8

_MM_DTYPE = os.environ.get("KERNEL_MM_DTYPE", "float32r")

_cache = {}


def _to_mm_np(a):
    """Cast a float32 host array to the numpy dtype of the matmul operands."""
    if _MM_DTYPE == "bfloat16":
        import ml_dtypes

        return np.ascontiguousarray(a.astype(ml_dtypes.bfloat16))
    return np.ascontiguousarray(a)


# --------------------------------------------------------------------------
# host-side G construction
# --------------------------------------------------------------------------
def _build_tables_np(dlnf):
    """Reference's per-chirp tables in numpy float32 (fallback path)."""
    dlnf = dlnf.astype(np.float32)
    beta = (2.0 * dlnf).astype(np.float32)
    small = np.abs(beta) < EPS
    beta_safe = np.where(small, np.float32(EPS), beta).astype(np.float32)
    e2b = np.exp(2.0 * beta_safe).astype(np.float32)

    tau = (2.0 * np.arange(KTAU, dtype=np.float32) / KTAU - 1.0).astype(np.float32)
    t_source = np.log(
        1.0 + (tau[None, :] + 1.0) / 2.0 * (e2b[:, None] - 1.0)
    ).astype(np.float32)
    t_source = (t_source / beta_safe[:, None] - 1.0).astype(np.float32)
    t_source = np.where(small[:, None], tau[None, :], t_source)

    tau_mid = np.float32(2.0 * (KTAU // 2) / KTAU - 1.0)
    t_mid = (
        np.log(1.0 + (tau_mid + 1.0) / 2.0 * (e2b - 1.0)) / beta_safe - 1.0
    ).astype(np.float32)
    t_mid = np.where(small, tau_mid, t_mid)

    jac = np.exp(-beta_safe[:, None] * (t_source - t_mid[:, None])).astype(np.float32)
    jac = np.where(small[:, None], np.float32(1.0), jac)

    idx = (np.float32(K / 2.0) * (t_source + 1.0)).astype(np.float32)
    idx_lo = np.clip(idx.astype(np.int32), 0, K - 2)
    frac = (idx - idx_lo.astype(np.float32)).astype(np.float32)
    return idx_lo, frac, jac


def _build_tables(dlnf):
    """Per-chirp tables, computed with jax on the CPU backend so the f32
    transcendentals (log/exp) match the reference bit-for-bit."""
    try:
        import jax
        import jax.numpy as jnp

        cpu = jax.devices("cpu")[0]
    except Exception:
        return _build_tables_np(dlnf)

    with jax.default_device(cpu):
        beta = 2.0 * jnp.asarray(dlnf, dtype=jnp.float32)
        small = jnp.abs(beta) < EPS
        beta_safe = jnp.where(small, EPS, beta)
        e2b = jnp.exp(2.0 * beta_safe)

        tau = 2.0 * jnp.arange(KTAU, dtype=jnp.float32) / KTAU - 1.0
        t_source = (
            jnp.log(1.0 + (tau[None, :] + 1.0) / 2.0 * (e2b[:, None] - 1.0))
            / beta_safe[:, None]
            - 1.0
        )
        t_source = jnp.where(small[:, None], tau[None, :], t_source)

        tau_mid = 2.0 * (KTAU // 2) / KTAU - 1.0
        t_mid = (
            jnp.log(1.0 + (tau_mid + 1.0) / 2.0 * (e2b - 1.0)) / beta_safe - 1.0
        )
        t_mid = jnp.where(small, tau_mid, t_mid)

        jac = jnp.exp(-beta_safe[:, None] * (t_source - t_mid[:, None]))
        jac = jnp.where(small[:, None], 1.0, jac)

        idx = (K / 2.0) * (t_source + 1.0)
        idx_lo = jnp.clip(idx.astype(jnp.int32), 0, K - 2)
        frac = idx - idx_lo.astype(jnp.float32)
    return np.asarray(idx_lo), np.asarray(frac), np.asarray(jac)


def _build_G(dlnf):
    """G[d, k, f2] f32: fused hann * lerp-resample * jac * rfft operator."""
    nd = dlnf.shape[0]
    idx_lo, frac, jac = _build_tables(dlnf)
    t = np.arange(KTAU, dtype=np.float64)
    f = np.arange(NF, dtype=np.float64)
    ang = 2.0 * np.pi * np.outer(t, f) / KTAU
    Wre = np.cos(ang)
    Wim = -np.sin(ang)
    n = np.arange(K, dtype=np.float32)
    hann = (0.5 * (1.0 - np.cos(2.0 * np.pi * n / K))).astype(np.float32)

    G = np.zeros((nd, K, FW), dtype=np.float64)
    for d in range(nd):
        c_lo = (jac[d] * (1.0 - frac[d])).astype(np.float64)
        c_hi = (jac[d] * frac[d]).astype(np.float64)
        Gre = np.zeros((K, NF))
        Gim = np.zeros((K, NF))
        np.add.at(Gre, idx_lo[d], c_lo[:, None] * Wre)
        np.add.at(Gim, idx_lo[d], c_lo[:, None] * Wim)
        np.add.at(Gre, idx_lo[d] + 1, c_hi[:, None] * Wre)
        np.add.at(Gim, idx_lo[d] + 1, c_hi[:, None] * Wim)
        G[d, :, 0::2] = Gre
        G[d, :, 1::2] = Gim
    G *= hann[None, :, None].astype(np.float64)
    G[:, :, FW - 1] = 0.0  # rfft Nyquist imag is exactly 0
    return G.astype(np.float32)


# --------------------------------------------------------------------------
# device program
# --------------------------------------------------------------------------
def _build_nc(iters=1):
    import concourse.bass as bass
    import concourse.bacc as bacc
    import concourse.mybir as mybir
    from concourse import tile

    mm_dt = {
        "float32r": mybir.dt.float32r,
        "float32": mybir.dt.float32,
        "bfloat16": mybir.dt.bfloat16,
    }[_MM_DTYPE]
    f32 = mybir.dt.float32

    nc = bacc.Bacc("TRN2", target_bir_lowering=False, debug=False)

    xt_d = nc.dram_tensor("xt", [128, B, MQ, 4], mm_dt, kind="ExternalInput")
    g_d = nc.dram_tensor("g", [128, D_PER, KC, FW], mm_dt, kind="ExternalInput")
    out_d = nc.dram_tensor("out", [B, NW, D_PER, FW], f32, kind="ExternalOutput")

    def body(nc, tc, xpool, gpool, spool, ppool):
        x_sb = xpool.tile([128, B, MQ, 4], mm_dt, name="x_sb")
        g_sb = gpool.tile([128, D_PER, KC, FW], mm_dt, name="g_sb")

        # split loads so the first matmuls don't wait for everything
        for b in range(B):
            nc.sync.dma_start(x_sb[:, b], xt_d[:, b])
        for fc in range(NFCH):
            fs = slice(fc * FCH, (fc + 1) * FCH)
            nc.sync.dma_start(g_sb[:, :, :, fs], g_d[:, :, :, fs])

        order = os.environ.get("KERNEL_ORDER", "fc")
        for b in range(B):
            w0 = 0
            for rc, cnt in enumerate(RC_SIZES):
                st = spool.tile([128, D_PER, FW], f32, name="st")
                if order == "fc":
                    # fc-outer: 2 PSUM banks per pass, weight reloaded per fc
                    for fc in range(NFCH):
                        fs = slice(fc * FCH, (fc + 1) * FCH)
                        ps = [
                            ppool.tile([128, FCH], f32, name="ps", tag="ps")
                            for _ in range(D_PER)
                        ]
                        for kc in range(KC):
                            # window w, chunk kc -> m = 4*w + kc
                            q, r = divmod(kc, 4)
                            lhsT = x_sb[:, b, w0 + q : w0 + q + cnt, r]
                            for d in range(D_PER):
                                nc.tensor.matmul(
                                    ps[d][:cnt],
                                    lhsT,
                                    g_sb[:, d, kc, fs],
                                    start=(kc == 0),
                                    stop=(kc == KC - 1),
                                )
                        nc.vector.tensor_copy(st[:cnt, 0, fs], ps[0][:cnt])
                        nc.scalar.copy(st[:cnt, 1, fs], ps[1][:cnt])
                else:
                    # kc-outer: one weight load feeds all 6 (d, fc) matmuls
                    ps = [
                        [
                            ppool.tile([128, FCH], f32, name="ps", tag="ps")
                            for _ in range(NFCH)
                        ]
                        for _ in range(D_PER)
                    ]
                    for kc in range(KC):
                        q, r = divmod(kc, 4)
                        lhsT = x_sb[:, b, w0 + q : w0 + q + cnt, r]
                        for d in range(D_PER):
                            for fc in range(NFCH):
                                fs = slice(fc * FCH, (fc + 1) * FCH)
                                nc.tensor.matmul(
                                    ps[d][fc][:cnt],
                                    lhsT,
                                    g_sb[:, d, kc, fs],
                                    start=(kc == 0),
                                    stop=(kc == KC - 1),
                                )
                    for fc in range(NFCH):
                        fs = slice(fc * FCH, (fc + 1) * FCH)
                        nc.vector.tensor_copy(st[:cnt, 0, fs], ps[0][fc][:cnt])
                        nc.scalar.copy(st[:cnt, 1, fs], ps[1][fc][:cnt])
                nc.sync.dma_start(out_d[b, w0 : w0 + cnt], st[:cnt])
                w0 += cnt

    loop = iters > 1 and os.environ.get("KERNEL_LOOP", "1") == "1"
    with tile.TileContext(nc) as tc:
        with (
            tc.tile_pool(name="xsb", bufs=1) as xpool,
            tc.tile_pool(name="gsb", bufs=1) as gpool,
            tc.tile_pool(name="stage", bufs=3) as spool,
            tc.tile_pool(name="psum", bufs=8, space="PSUM") as ppool,
        ):
            if loop:
                with tc.For_i(0, iters, 1):
                    body(nc, tc, xpool, gpool, spool, ppool)
            else:
                for _ in range(iters):
                    body(nc, tc, xpool, gpool, spool, ppool)

    nc.compile()
    return nc


def _get_nc(iters=1):
    key = ("nc", iters)
    if key not in _cache:
        _cache[key] = _build_nc(iters)
    return _cache[key]


# --------------------------------------------------------------------------
# entry point
# --------------------------------------------------------------------------
def _prep_arrays(x, dlnf):
    """Host prep: G matrices + transposed/sharded device input arrays."""
    x = np.asarray(x, dtype=np.float32)
    dlnf = np.asarray(dlnf, dtype=np.float32)
    G = _build_G(dlnf)                                     # (16, 1024, 1026)
    # g layout per core: [128, D_PER, KC, FW]
    g_all = _to_mm_np(G.reshape(D, KC, 128, FW).transpose(2, 0, 1, 3))
    # xt layout: [128, B, MQ, 4]; xt[p, b, mq, r] = x[b, 128*(4*mq+r) + p]
    xt = _to_mm_np(x.reshape(B, MQ, 4, 128).transpose(3, 0, 1, 2))
    in_maps = [
        {"xt": xt, "g": np.ascontiguousarray(g_all[:, c * D_PER : (c + 1) * D_PER])}
        for c in range(NCORES)
    ]
    return in_maps


def _get_runner(iters):
    """Build (once) a jitted multi-core executable for the iters-body program.

    Mirrors bass2jax.run_bass_via_pjrt's multi-core branch, but caches the
    jitted callable so repeat kernel() calls skip retrace/relower/recompile.
    """
    key = ("runner", iters)
    if key in _cache:
        return _cache[key]

    import jax
    from jax.experimental.shard_map import shard_map
    from jax.sharding import Mesh, PartitionSpec

    from concourse import bass2jax as b2j
    import concourse.mybir as mybir

    b2j.install_neuronx_cc_hook()
    nc = _get_nc(iters)
    partition_name = (
        nc.partition_id_tensor.name if nc.partition_id_tensor else None
    )

    in_names, out_names, out_avals, zero_outs = [], [], [], []
    for alloc in nc.m.functions[0].allocations:
        if not isinstance(alloc, mybir.MemoryLocationSet):
            continue
        name = alloc.memorylocations[0].name
        if alloc.kind == "ExternalInput":
            if name != partition_name:
                in_names.append(name)
        elif alloc.kind == "ExternalOutput":
            out_names.append(name)
            shape = tuple(alloc.tensor_shape)
            dtype = mybir.dt.np(alloc.dtype)
            out_avals.append(jax.core.ShapedArray(shape, dtype))
            zero_outs.append(np.zeros(shape, dtype))
    n_params = len(in_names)
    all_names = in_names + out_names
    if partition_name is not None:
        all_names = all_names + [partition_name]

    def _body(*args):
        operands = list(args)
        if partition_name is not None:
            operands.append(b2j.partition_id_tensor())
        outs = b2j._bass_exec_p.bind(
            *operands,
            out_avals=tuple(out_avals),
            in_names=tuple(all_names),
            out_names=tuple(out_names),
            lowering_input_output_aliases=(),
            sim_require_finite=True,
            sim_require_nnan=True,
            nc=nc,
        )
        return tuple(outs)

    devices = jax.devices()[:NCORES]
    mesh = Mesh(np.asarray(devices), ("core",))
    nin = n_params + len(zero_outs)
    sharded = jax.jit(
        shard_map(
            _body,
            mesh=mesh,
            in_specs=(PartitionSpec("core"),) * nin,
            out_specs=(PartitionSpec("core"),) * len(out_names),
            check_rep=False,
        ),
        keep_unused=True,
    )

    def call(in_maps):
        concat_in = [
            np.concatenate([in_maps[c][name] for c in range(NCORES)], axis=0)
            for name in in_names
        ] + [
            np.zeros((NCORES * z.shape[0], *z.shape[1:]), z.dtype)
            for z in zero_outs
        ]
        out_arrs = sharded(*concat_in)
        jax.block_until_ready(out_arrs)
        return [
            {
                name: np.asarray(out_arrs[i]).reshape(
                    NCORES, *out_avals[i].shape
                )[c]
                for i, name in enumerate(out_names)
            }
            for c in range(NCORES)
        ]

    _cache[key] = call
    return call


def kernel(x, dlnf, n_hann_splits=1, **_unused):
    in_maps = _prep_arrays(x, dlnf)
    iters = int(os.environ.get("KERNEL_ITERS", "1"))
    try:
        call = _get_runner(iters)
        results = call(in_maps)
    except Exception:
        # robust fallback: the reference implementation of the SPMD runner
        from concourse.bass_utils import run_bass_kernel_spmd

        nc = _get_nc(iters)
        res = run_bass_kernel_spmd(nc, in_maps, core_ids=list(range(NCORES)))
        results = res.results

    # assemble: per-core (B, NW, D_PER, FW) f32 -> (B, NW, D, NF) c64
    per_core = [
        r["out"].view(np.complex64).reshape(B, NW, D_PER, NF) for r in results
    ]
    out = np.concatenate(per_core, axis=2)
    return out


# --------------------------------------------------------------------------
# benchmarking: jit once, time repeated executions (no retrace/relower)
# --------------------------------------------------------------------------
def prepare_bench(x, dlnf, iters):
    """Returns run() -> wall seconds for one execution of the iters-body NEFF."""
    import time

    import jax
    from jax.experimental.shard_map import shard_map
    from jax.sharding import Mesh, PartitionSpec

    from concourse import bass2jax as b2j
    import concourse.mybir as mybir

    b2j.install_neuronx_cc_hook()

    x = np.asarray(x, dtype=np.float32)
    G = _build_G(np.asarray(dlnf, dtype=np.float32))
    g_all = _to_mm_np(G.reshape(D, KC, 128, FW).transpose(2, 0, 1, 3))
    xt = _to_mm_np(x.reshape(B, MQ, 4, 128).transpose(3, 0, 1, 2))
    in_maps = [
        {"xt": xt, "g": np.ascontiguousarray(g_all[:, c * D_PER : (c + 1) * D_PER])}
        for c in range(NCORES)
    ]

    nc = _get_nc(iters)
    assert nc.dbg_addr is None
    partition_name = (
        nc.partition_id_tensor.name if nc.partition_id_tensor else None
    )

    in_names, out_names, out_avals, zero_outs = [], [], [], []
    for alloc in nc.m.functions[0].allocations:
        if not isinstance(alloc, mybir.MemoryLocationSet):
            continue
        name = alloc.memorylocations[0].name
        if alloc.kind == "ExternalInput":
            if name != partition_name:
                in_names.append(name)
        elif alloc.kind == "ExternalOutput":
            out_names.append(name)
            shape = tuple(alloc.tensor_shape)
            dtype = mybir.dt.np(alloc.dtype)
            out_avals.append(jax.core.ShapedArray(shape, dtype))
            zero_outs.append(np.zeros(shape, dtype))
    n_params = len(in_names)
    all_names = in_names + out_names
    if partition_name is not None:
        all_names = all_names + [partition_name]

    def _body(*args):
        operands = list(args)
        if partition_name is not None:
            operands.append(b2j.partition_id_tensor())
        outs = b2j._bass_exec_p.bind(
            *operands,
            out_avals=tuple(out_avals),
            in_names=tuple(all_names),
            out_names=tuple(out_names),
            lowering_input_output_aliases=(),
            sim_require_finite=True,
            sim_require_nnan=True,
            nc=nc,
        )
        return tuple(outs)

    devices = jax.devices()[:NCORES]
    mesh = Mesh(np.asarray(devices), ("core",))
    nin = n_params + len(zero_outs)
    sharded = jax.jit(
        shard_map(
            _body,
            mesh=mesh,
            in_specs=(PartitionSpec("core"),) * nin,
            out_specs=(PartitionSpec("core"),) * len(out_names),
            check_rep=False,
        ),
        keep_unused=True,
    )
    concat_in = [
        np.concatenate([in_maps[c][name] for c in range(NCORES)], axis=0)
        for name in in_names
    ] + [np.zeros((NCORES * z.shape[0], *z.shape[1:]), z.dtype) for z in zero_outs]
    concat_in = [jax.device_put(a) for a in concat_in]

    # warm compile
    out = sharded(*concat_in)
    jax.block_until_ready(out)

    def run():
        t0 = time.perf_counter()
        o = sharded(*concat_in)
        jax.block_until_ready(o)
        return time.perf_counter() - t0

    return run


if __name__ == "__main__":
    rng = np.random.default_rng(0)
    x = rng.standard_normal((B, N), dtype=np.float32)
    dlnf = rng.uniform(-0.5, 0.5, size=(D,)).astype(np.float32)
    out = kernel(x, dlnf, 1)
    print("out:", out.shape, out.dtype)
